# revision 24
# baseline (speedup 1.0000x reference)
"""GAT layer (2 steps) on 8 Trainium2 NeuronCores via Bass/Tile.

Strategy (edge partitioning by destination, per sharding hint):
  - Nodes padded to 10240 = 8 dev x 10 blocks x 128. Device d owns dst blocks
    10d..10d+10 and all edges pointing into them (host groups edges by dst).
  - Each device computes the full projected-feature table for all nodes
    (replicated compute), writes it to DRAM as 768-B rows
    [h fp8e4m3 512B | el f32 16B | er f32 16B | pad], then gathers rows by
    edge src via dma_gather.
  - dma_gather descriptor generation is the serial bottleneck on one SWDGE
    queue (~7.5 ns/idx on a Q7 pair); gathers are round-robined over all 4
    SWDGE queues so up to 4 Q7 pairs generate concurrently and the phase
    becomes DMA-bandwidth-bound instead.
  - Softmax over incoming edges is shift-invariant, so segment_max is replaced
    by a constant shift; normalization happens per dst node after aggregation.
  - Aggregation = one-hot matmul: out[n,:] += sum_e Q[e,n] * (ex_e * h_src_e).
    Q (edges->nodes) and QT (nodes->edges, for broadcasting the per-dst er
    term) are graph-static one-hot fp8 tiles built on the host and streamed
    per chunk; denominators ride along as an extra tiny matmul on Q.
  - Nodes are assigned to blocks with a degree-balancing permutation so all
    blocks carry nearly equal edge counts (minimizes chunk padding).
  - Between the two GAT steps: on-device AllGather of the transposed new x
    in bf16.
"""
import os
import sys

sys.path.insert(0, "/opt/trn_rl_repo")

import numpy as np

LAST_RES = None

N = 10000
E = 320000
F = 128
H = 4
HF = H * F  # 512
NDEV = 8
NPAD = 10240
NBLK = 80
NBLK_DEV = 10
DEVN = NBLK_DEV * 128  # 1280
ROWB = 768  # bytes per table row: h fp8 @0:512, el f16 @512:520, er f16 @520:528
SENT = NPAD  # sentinel row index for pad edges
C_SHIFT = 4.0  # constant softmax shift (replaces segment_max)
NEG_SLOPE = 0.2
ONE_F8E4 = 0x38  # 1.0 in fp8 e4m3

_CACHE = {}


# ---------------------------------------------------------------- host prep
def _prep_graph(src, dst):
    # degree-balanced node -> slot permutation: assign nodes to the 80 blocks
    # so per-block edge counts are nearly equal (minimizes chunk padding).
    deg = np.bincount(dst, minlength=N)
    order_n = np.argsort(-deg, kind="stable")
    blk_load = np.zeros(NBLK, np.int64)
    blk_fill = np.zeros(NBLK, np.int64)
    slot = np.empty(NPAD, np.int64)
    node_of_slot = np.full(NPAD, -1, np.int64)
    import heapq
    heap = [(0, 0, b) for b in range(NBLK)]
    heapq.heapify(heap)
    for n in order_n:
        while True:
            load, fill, b = heapq.heappop(heap)
            if fill < 128 and fill == blk_fill[b] and load == blk_load[b]:
                break
        s = 128 * b + fill
        slot[n] = s
        node_of_slot[s] = n
        blk_load[b] += deg[n]
        blk_fill[b] += 1
        if blk_fill[b] < 128:
            heapq.heappush(heap, (int(blk_load[b]), int(blk_fill[b]), b))
    # pad nodes (no edges) fill remaining slots
    free_slots = np.where(node_of_slot < 0)[0]
    for s, vn in zip(free_slots, range(N, N + len(free_slots))):
        node_of_slot[s] = vn
    sdst = slot[dst]
    ssrc = slot[src]
    order = np.argsort(sdst, kind="stable")
    s_src = ssrc[order]
    s_dst = sdst[order]
    blk = s_dst // 128
    counts = np.bincount(blk, minlength=NBLK)
    maxcnt = int(counts.max())
    nch = max(2, 2 * ((maxcnt + 255) // 256))  # even chunk count per block
    ebpad = nch * 128
    nhalf = nch // 2
    nipc = nhalf * 128  # idxs per gather call

    starts = np.zeros(NBLK + 1, np.int64)
    np.cumsum(counts, out=starts[1:])

    # per-block padded arrays
    gidx = np.full((NBLK, ebpad), SENT, np.int64)
    dloc = np.zeros((NBLK, ebpad), np.int64)
    for b in range(NBLK):
        lo, hi = starts[b], starts[b + 1]
        cnt = hi - lo
        gidx[b, :cnt] = s_src[lo:hi]
        dloc[b, :cnt] = s_dst[lo:hi] - 128 * b

    per_core = []
    for d in range(NDEV):
        bsl = slice(NBLK_DEV * d, NBLK_DEV * (d + 1))
        g = gidx[bsl]  # [10, ebpad]
        dl = dloc[bsl]

        # big gather idx tiles: [10*2 calls, 128, nipc//16] int16
        bigidx = np.zeros((NBLK_DEV * 2, 128, nipc // 16), np.int16)
        gi = g.reshape(NBLK_DEV, 2, nipc)
        for b in range(NBLK_DEV):
            for hf_ in range(2):
                v = gi[b, hf_]  # [nipc]
                t = v.reshape(nipc // 16, 16).T.astype(np.int16)  # [16, cols]
                bigidx[2 * b + hf_] = np.tile(t, (8, 1))

        # er gather idxs: all own rows d*1280 .. d*1280+1280 -> [128, 80]
        v = (DEVN * d + np.arange(DEVN)).astype(np.int16)
        eridx = np.tile(v.reshape(DEVN // 16, 16).T, (8, 1))

        # packed one-hot tiles [10*nch, 128, 256] fp8e4m3 (as uint8 bytes):
        #   cols 0:128  = QT:  qtq[k, n, e]    = dloc(k, e) == n
        #   cols 128:256 = Q:  qtq[k, e, 128+n] = dloc(k, e) == n
        dl3 = dl.reshape(NBLK_DEV, nch, 128)  # [b, c, p]
        nk = NBLK_DEV * nch
        qtq = np.zeros((nk, 128, 256), np.uint8)
        ch_idx = np.repeat(np.arange(nk), 128)
        p_idx = np.tile(np.arange(128), nk)
        n_idx = dl3.reshape(-1)
        qtq[ch_idx, n_idx, p_idx] = ONE_F8E4
        qtq[ch_idx, p_idx, 128 + n_idx] = ONE_F8E4

        per_core.append(dict(bigidx=bigidx, eridx=eridx, qtq=qtq))
    return per_core, nch, slot, node_of_slot


def _build(nch, alpha):
    import concourse.bass as bass
    import concourse.tile as tile
    from concourse import bacc, mybir

    f32 = mybir.dt.float32
    f32r = mybir.dt.float32r
    f16 = mybir.dt.float16
    bf16 = mybir.dt.bfloat16
    f8 = mybir.dt.float8e4
    i16 = mybir.dt.int16
    u8 = mybir.dt.uint8
    nhalf = nch // 2
    nipc = nhalf * 128
    icols = nipc // 16
    CA = float((1.0 - alpha) / H)

    nc = bacc.Bacc("TRN2", target_bir_lowering=False, debug=False,
                   num_devices=NDEV, num_swdge_queues=4)

    # ---- params (shared across cores unless noted)
    xT0_p = nc.declare_dram_parameter("xT0", [128, NPAD], bf16, isOutput=False)
    W_p = nc.declare_dram_parameter("Wm", [128, HF], f32, isOutput=False)
    ALR_p = nc.declare_dram_parameter("ALR", [128, 2 * H], f32, isOutput=False)
    x0b_p = nc.declare_dram_parameter("x0b", [DEVN, F], f32, isOutput=False)  # per-core
    ident_p = nc.declare_dram_parameter("ident32", [128, 128], f32, isOutput=False)
    bigidx_p = nc.declare_dram_parameter("bigidx", [NBLK_DEV * 2, 128, icols], i16, isOutput=False)  # per-core
    eridx_p = nc.declare_dram_parameter("eridx", [128, DEVN // 16], i16, isOutput=False)  # per-core
    qtq_p = nc.declare_dram_parameter("qtq", [NBLK_DEV * nch, 128, 256], u8, isOutput=False)  # per-core
    sent_p = nc.declare_dram_parameter("sentrow", [1, ROWB], u8, isOutput=False)
    out_p = nc.declare_dram_parameter("outx", [DEVN, F], f32, isOutput=True)  # per-core

    # ---- internal DRAM
    h_table = nc.dram_tensor("h_table", [NPAD + 16, ROWB], u8)
    xt_own0 = nc.dram_tensor("xt_own0", [128, 6 * 128], bf16)
    xt_own1 = nc.dram_tensor("xt_own1", [128, 4 * 128], bf16)
    ag0 = nc.dram_tensor("ag0", [NDEV, 128, 6 * 128], bf16, addr_space="Shared")
    ag1 = nc.dram_tensor("ag1", [NDEV, 128, 4 * 128], bf16, addr_space="Shared")

    # queue 0's Q7 pair appears to host the Pool sequencer: gathers on it hold
    # the engine for their full descriptor-generation time, while queues 1-3
    # retire fast and generate asynchronously. Rotate over 1..3 only.
    gq = [0]

    def next_q():
        q = gq[0]
        gq[0] = (gq[0] + 1) % 3
        return 1 + q

    from contextlib import ExitStack
    with tile.TileContext(nc) as tc, ExitStack() as ctx:
        cpool = ctx.enter_context(tc.tile_pool(name="consts", bufs=1))
        gpool = ctx.enter_context(tc.tile_pool(name="gather", bufs=8))
        stpool = ctx.enter_context(tc.tile_pool(name="stage", bufs=4))
        xtpool = ctx.enter_context(tc.tile_pool(name="xt", bufs=4))
        qtpool = ctx.enter_context(tc.tile_pool(name="qt", bufs=6))
        mpool = ctx.enter_context(tc.tile_pool(name="msg", bufs=6))
        apool = ctx.enter_context(tc.tile_pool(name="attn", bufs=6))
        epool = ctx.enter_context(tc.tile_pool(name="epi", bufs=2))
        pbig = ctx.enter_context(tc.tile_pool(name="pbig", bufs=3, space="PSUM"))
        psm = ctx.enter_context(tc.tile_pool(name="psm", bufs=3, space="PSUM"))
        per = ctx.enter_context(tc.tile_pool(name="per", bufs=2, space="PSUM"))

        # ---- load constants
        W_sb = cpool.tile([128, HF], f32, tag="W")
        nc.sync.dma_start(out=W_sb[:], in_=W_p[:])
        W_r = cpool.tile([128, HF], bf16, tag="Wr")
        nc.vector.tensor_copy(out=W_r[:], in_=W_sb[:])
        ALR_sb = cpool.tile([128, 2 * H], f32, tag="ALR")
        nc.sync.dma_start(out=ALR_sb[:], in_=ALR_p[:])
        ALR_r = cpool.tile([128, 2 * H], bf16, tag="ALRr")
        nc.vector.tensor_copy(out=ALR_r[:], in_=ALR_sb[:])
        ident_sb = cpool.tile([128, 128], f32, tag="ident")
        nc.sync.dma_start(out=ident_sb[:], in_=ident_p[:])
        identB = cpool.tile([128, 128], f16, tag="identB")
        nc.vector.tensor_copy(out=identB[:], in_=ident_sb[:])
        bigidx_sb = cpool.tile([128, NBLK_DEV * 2 * icols], i16, tag="bigidx")
        for k in range(NBLK_DEV * 2):
            nc.sync.dma_start(
                out=bigidx_sb[:, k * icols:(k + 1) * icols], in_=bigidx_p[k]
            )
        eridx_sb = cpool.tile([128, DEVN // 16], i16, tag="eridx")
        nc.sync.dma_start(out=eridx_sb[:], in_=eridx_p[:])
        shift_sb = cpool.tile([128, 1], f32, tag="shift")
        nc.vector.memset(shift_sb[:], -C_SHIFT)
        sent_sb = cpool.tile([1, ROWB], u8, tag="sent")
        nc.sync.dma_start(out=sent_sb[:], in_=sent_p[:])
        nc.sync.dma_start(out=h_table[SENT:SENT + 1, :], in_=sent_sb[:])

        for step in range(2):
            # ================= H phase: build h_ext table for all nodes
            for c4 in range(NBLK // 4):
                xt4 = xtpool.tile([128, 4, 128], bf16, tag="xt")
                if step == 0:
                    nc.sync.dma_start(
                        out=xt4[:], in_=xT0_p[:, 512 * c4:512 * (c4 + 1)].rearrange(
                            "p (c q) -> p c q", c=4)
                    )
                else:
                    # 4 consecutive chunks may straddle an ag rank boundary only
                    # when NBLK_DEV % 4 != 0; NBLK_DEV=10 -> straddles every other
                    # group, so split into two 2-chunk loads (2 | 2).
                    for half2 in range(2):
                        c2 = 4 * c4 + 2 * half2
                        r2, cc2 = c2 // NBLK_DEV, c2 % NBLK_DEV
                        if cc2 < 6:
                            ag_src = ag0[r2, :, 128 * cc2:128 * (cc2 + 2)]
                        else:
                            ag_src = ag1[r2, :, 128 * (cc2 - 6):128 * (cc2 - 4)]
                        nc.sync.dma_start(
                            out=xt4[:, 2 * half2:2 * half2 + 2, :],
                            in_=ag_src.rearrange("p (c q) -> p c q", c=2),
                        )
                for j2 in range(2):
                    stage = stpool.tile([128, 2, ROWB], u8, tag="stage")
                    for jj in range(2):
                        c = 4 * c4 + 2 * j2 + jj
                        xt_sl = xt4[:, 2 * j2 + jj, :]
                        h_ps = pbig.tile([128, HF], f32, tag="big")
                        nc.tensor.matmul(out=h_ps[:], lhsT=xt_sl, rhs=W_r[:],
                                         start=True, stop=True)
                        e_ps = psm.tile([128, 128], f32, tag="sm")
                        nc.tensor.matmul(
                            out=e_ps[:, 0:2 * H], lhsT=xt_sl, rhs=ALR_r[:],
                            start=True, stop=True
                        )
                        if jj == 0:
                            nc.scalar.activation(
                                out=stage[:, jj, 0:HF].bitcast(f8), in_=h_ps[:],
                                func=mybir.ActivationFunctionType.Copy
                            )
                        else:
                            nc.vector.tensor_copy(
                                out=stage[:, jj, 0:HF].bitcast(f8), in_=h_ps[:]
                            )
                        nc.vector.tensor_copy(
                            out=stage[:, jj, HF:HF + 16].bitcast(f16), in_=e_ps[:, 0:2 * H]
                        )
                    c0 = 4 * c4 + 2 * j2
                    nc.sync.dma_start(
                        out=h_table[128 * c0:128 * (c0 + 2), :].rearrange(
                            "(j p) w -> p j w", p=128),
                        in_=stage[:],
                    )

            # ================= AGG phase: own blocks
            erg = epool.tile([128, NBLK_DEV, 256], u8, tag="erg")
            nc.gpsimd.dma_gather(
                out_ap=erg[:],
                in_ap=h_table[:, HF:HF + 256],
                idxs_ap=eridx_sb[:],
                num_idxs=DEVN,
                num_idxs_reg=DEVN,
                elem_size=256,
                elem_step=ROWB,
                single_packet=False,
                queue_num=next_q(),
            )
            def attention(b):
                halves = []
                for hf_ in range(2):
                    G = gpool.tile([128, nhalf, ROWB], u8, tag="G")
                    call = 2 * b + hf_
                    nc.gpsimd.dma_gather(
                        out_ap=G[:],
                        in_ap=h_table[0:NPAD + 16, 0:ROWB],
                        idxs_ap=bigidx_sb[:, call * icols:(call + 1) * icols],
                        num_idxs=nipc,
                        num_idxs_reg=nipc,
                        elem_size=ROWB,
                        elem_step=ROWB,
                        single_packet=False,
                        queue_num=next_q(),
                    )
                    k0 = b * nch + hf_ * nhalf
                    qtq_t = qtpool.tile([128, nhalf, 256], u8, tag="qt")
                    nc.sync.dma_start(
                        out=qtq_t[:],
                        in_=qtq_p[k0:k0 + nhalf].rearrange("c p w -> p c w"),
                    )
                    er_ps = per.tile([128, 4 * nhalf], f32, tag="er")
                    for cc in range(nhalf):
                        nc.tensor.matmul(
                            out=er_ps[:, 4 * cc:4 * cc + 4],
                            lhsT=qtq_t[:, cc, 0:128].bitcast(f8),
                            rhs=erg[:, b, 8:16].bitcast(f16),
                            start=True, stop=True,
                        )
                    z = apool.tile([128, 4 * nhalf], f32, tag="z")
                    nc.vector.tensor_tensor(
                        out=z[:], in0=G[:, :, HF:HF + 8].bitcast(f16), in1=er_ps[:],
                        op=mybir.AluOpType.add
                    )
                    v = apool.tile([128, 4 * nhalf], f32, tag="v")
                    nc.vector.tensor_scalar(
                        out=v[:], in0=z[:], scalar1=NEG_SLOPE, scalar2=None,
                        op0=mybir.AluOpType.mult,
                    )
                    w = apool.tile([128, 4 * nhalf], f32, tag="w")
                    nc.vector.tensor_tensor(
                        out=w[:], in0=z[:], in1=v[:], op=mybir.AluOpType.max
                    )
                    ex32 = apool.tile([128, 4 * nhalf], f32, tag="ex32")
                    nc.scalar.activation(
                        out=ex32[:], in_=w[:], func=mybir.ActivationFunctionType.Exp,
                        bias=shift_sb[:, 0:1],
                    )
                    ex16 = apool.tile([128, 4 * nhalf], f16, tag="ex")
                    nc.scalar.activation(
                        out=ex16[:], in_=ex32[:],
                        func=mybir.ActivationFunctionType.Copy,
                    )
                    halves.append((G, qtq_t, ex16, ex32))
                return halves

            def aggregation(b, halves, step):
                out_ps = pbig.tile([128, HF], f32, tag="big")
                den_ps = psm.tile([128, 128], f32, tag="sm")
                for hf_, (G, qtq_t, ex16, ex32) in enumerate(halves):
                    # denominator matmuls first: their rhs (ex16) is ready early
                    for cc in range(nhalf):
                        cg = hf_ * nhalf + cc
                        nc.tensor.matmul(
                            out=den_ps[:, 0:H], lhsT=qtq_t[:, cc, 128:256].bitcast(f8),
                            rhs=ex16[:, 4 * cc:4 * cc + 4],
                            start=(cg == 0), stop=(cg == nch - 1), skip_group_check=True,
                        )
                    for cc in range(nhalf):
                        cg = hf_ * nhalf + cc
                        msg = mpool.tile([128, H, F], f16, tag="msg")
                        if cg % 2 == 0:
                            # DVE path: broadcast multiply (~600 ns)
                            nc.vector.tensor_tensor(
                                out=msg[:],
                                in0=G[:, cc, 0:HF].bitcast(f8).rearrange(
                                    "p (h f) -> p h f", h=H),
                                in1=ex16[:, 4 * cc:4 * cc + 4, None].to_broadcast([128, H, F]),
                                op=mybir.AluOpType.mult,
                            )
                        else:
                            # Act path: per-head scaled copies; for a fixed
                            # head the ex factor is per-partition (per-edge),
                            # which is Act's native scale operand
                            for hd in range(H):
                                nc.scalar.activation(
                                    out=msg[:, hd],
                                    in_=G[:, cc, F * hd:F * (hd + 1)].bitcast(f8),
                                    func=mybir.ActivationFunctionType.Copy,
                                    scale=ex32[:, 4 * cc + hd:4 * cc + hd + 1],
                                )
                        nc.tensor.matmul(
                            out=out_ps[:], lhsT=qtq_t[:, cc, 128:256].bitcast(f8),
                            rhs=msg[:].rearrange("p h f -> p (h f)"),
                            start=(cg == 0), stop=(cg == nch - 1), skip_group_check=True,
                        )

                # ---- epilogue for block b
                den_sb = epool.tile([128, H], f32, tag="den")
                nc.vector.tensor_scalar(
                    out=den_sb[:], in0=den_ps[:, 0:H], scalar1=1e-30, scalar2=None,
                    op0=mybir.AluOpType.add,
                )
                rden = epool.tile([128, H], f32, tag="rden")
                nc.vector.reciprocal(out=rden[:], in_=den_sb[:])
                ms = []
                for hd in range(H):
                    m = epool.tile([128, F], f32, tag=f"m{hd}")
                    nc.scalar.activation(
                        out=m[:], in_=out_ps[:, F * hd:F * (hd + 1)],
                        func=mybir.ActivationFunctionType.Copy,
                        scale=rden[:, hd:hd + 1],
                    )
                    ms.append(m)
                a01 = epool.tile([128, F], f32, tag="a01")
                nc.vector.tensor_tensor(out=a01[:], in0=ms[0][:], in1=ms[1][:], op=mybir.AluOpType.add)
                a23 = epool.tile([128, F], f32, tag="a23")
                nc.vector.tensor_tensor(out=a23[:], in0=ms[2][:], in1=ms[3][:], op=mybir.AluOpType.add)
                macc = epool.tile([128, F], f32, tag="macc")
                nc.vector.tensor_tensor(out=macc[:], in0=a01[:], in1=a23[:], op=mybir.AluOpType.add)
                x0b_t = epool.tile([128, F], f32, tag="x0b")
                nc.sync.dma_start(out=x0b_t[:], in_=x0b_p[128 * b:128 * (b + 1), :])
                sc = epool.tile([128, F], f32, tag="sc")
                nc.vector.tensor_scalar(
                    out=sc[:], in0=macc[:], scalar1=CA, scalar2=None, op0=mybir.AluOpType.mult
                )
                outf = epool.tile([128, F], f32, tag="outf")
                nc.vector.tensor_tensor(out=outf[:], in0=sc[:], in1=x0b_t[:], op=mybir.AluOpType.add)
                if step == 0:
                    tp_ps = psm.tile([128, 128], f32, tag="sm")
                    nc.tensor.transpose(out=tp_ps[:], in_=outf[:], identity=ident_sb[:])
                    xtb = epool.tile([128, 128], bf16, tag="xtb")
                    nc.vector.tensor_copy(out=xtb[:], in_=tp_ps[:])
                    if b < 6:
                        nc.sync.dma_start(out=xt_own0[:, 128 * b:128 * (b + 1)], in_=xtb[:])
                    else:
                        nc.sync.dma_start(out=xt_own1[:, 128 * (b - 6):128 * (b - 5)], in_=xtb[:])
                    if b == 5:
                        # first 6 blocks done on every rank: gather them while
                        # the last 4 blocks are still aggregating
                        nc.gpsimd.collective_compute(
                            "AllGather",
                            bass.mybir.AluOpType.bypass,
                            replica_groups=[list(range(NDEV))],
                            ins=[xt_own0[:]],
                            outs=[ag0[:]],
                        )
                else:
                    nc.sync.dma_start(out=out_p[128 * b:128 * (b + 1), :], in_=outf[:])

            # software pipeline: attention for block b+1 is issued before the
            # heavy aggregation matmuls of block b so the tensor/DVE/Act
            # streams of consecutive blocks overlap
            pend = attention(0)
            for b in range(NBLK_DEV):
                nxt = attention(b + 1) if b + 1 < NBLK_DEV else None
                aggregation(b, pend, step)
                pend = nxt

            if step == 0:
                nc.gpsimd.collective_compute(
                    "AllGather",
                    bass.mybir.AluOpType.bypass,
                    replica_groups=[list(range(NDEV))],
                    ins=[xt_own1[:]],
                    outs=[ag1[:]],
                )

    nc.compile()
    return nc


# ---------------------------------------------------------------- entry point
def kernel(x, x0, src, dst, alpha, W, attn_l, attn_r, bias):
    x = np.asarray(x, np.float32)
    x0 = np.asarray(x0, np.float32)
    src = np.asarray(src).astype(np.int64)
    dst = np.asarray(dst).astype(np.int64)
    alpha = float(np.asarray(alpha))
    W = np.asarray(W, np.float32)
    attn_l = np.asarray(attn_l, np.float32)
    attn_r = np.asarray(attn_r, np.float32)
    bias = np.asarray(bias, np.float32)

    per_core, nch, slot, node_of_slot = _prep_graph(src, dst)

    key = (nch, round(alpha, 9))
    if key not in _CACHE:
        _CACHE[key] = _build(nch, alpha)
    nc = _CACHE[key]

    # shared host inputs
    xpad = np.zeros((NPAD, F), np.float32)
    real = node_of_slot < N
    xpad[real] = x[node_of_slot[real]]
    import ml_dtypes
    xT0 = np.ascontiguousarray(xpad.T).astype(ml_dtypes.bfloat16)  # [128, NPAD]
    ALR = np.zeros((128, 2 * H), np.float32)
    Wr = W.reshape(F, H, F)
    ALR[:, 0:H] = np.einsum("fhg,hg->fh", Wr, attn_l)
    ALR[:, H:2 * H] = np.einsum("fhg,hg->fh", Wr, attn_r)
    ident32 = np.eye(128, dtype=np.float32)
    bias_mean = bias.mean(axis=0)  # [F]
    x0b_full = np.zeros((NPAD, F), np.float32)
    x0b_full[real] = alpha * x0[node_of_slot[real]] + (1.0 - alpha) * bias_mean[None, :]
    sentrow = np.zeros((1, ROWB), np.uint8)
    sentrow[0, HF:HF + 8] = np.full(4, -60000.0, np.float16).view(np.uint8)

    from concourse.bass_utils import run_bass_kernel_spmd

    in_maps = []
    for d in range(NDEV):
        pc = per_core[d]
        in_maps.append({
            "xT0": xT0, "Wm": W, "ALR": ALR,
            "x0b": x0b_full[DEVN * d:DEVN * (d + 1)],
            "ident32": ident32, "bigidx": pc["bigidx"],
            "eridx": pc["eridx"], "qtq": pc["qtq"], "sentrow": sentrow,
        })
    global LAST_RES
    res = None
    for attempt in range(3):
        try:
            res = run_bass_kernel_spmd(
                nc, in_maps, list(range(NDEV)),
                trace=bool(os.environ.get("GAT_TRACE")),
            )
            break
        except Exception:
            if attempt == 2:
                raise
            import time as _time
            _time.sleep(2.0)
    LAST_RES = res
    out_slots = np.concatenate([r["outx"] for r in res.results], axis=0)
    return out_slots[slot[np.arange(N)]].astype(np.float32)


if __name__ == "__main__":
    rng = np.random.default_rng(0)
    x = rng.standard_normal((N, F), dtype=np.float32)
    x0 = rng.standard_normal((N, F), dtype=np.float32)
    src = rng.integers(0, N, E).astype(np.int32)
    dst = rng.integers(0, N, E).astype(np.int32)
    W = (rng.standard_normal((F, H * F)).astype(np.float32) / np.sqrt(F))
    al = (rng.standard_normal((H, F)).astype(np.float32) / np.sqrt(F))
    ar = (rng.standard_normal((H, F)).astype(np.float32) / np.sqrt(F))
    bias = np.zeros((H, F), np.float32)
    out = kernel(x=x, x0=x0, src=src, dst=dst, alpha=np.float32(0.1),
                 W=W, attn_l=al, attn_r=ar, bias=bias)
    print("out", out.shape, out.dtype, float(np.abs(out).max()))


# revision 25
# speedup vs baseline: 1.0624x; 1.0624x over previous
"""GAT layer (2 steps) on 8 Trainium2 NeuronCores via Bass/Tile.

Strategy (edge partitioning by destination, per sharding hint):
  - Nodes padded to 10240 = 8 dev x 10 blocks x 128. Device d owns dst blocks
    10d..10d+10 and all edges pointing into them (host groups edges by dst).
  - Each device computes the full projected-feature table for all nodes
    (replicated compute), writes it to DRAM as 768-B rows
    [h fp8e4m3 512B | el f32 16B | er f32 16B | pad], then gathers rows by
    edge src via dma_gather.
  - dma_gather descriptor generation is the serial bottleneck on one SWDGE
    queue (~7.5 ns/idx on a Q7 pair); gathers are round-robined over all 4
    SWDGE queues so up to 4 Q7 pairs generate concurrently and the phase
    becomes DMA-bandwidth-bound instead.
  - Softmax over incoming edges is shift-invariant, so segment_max is replaced
    by a constant shift; normalization happens per dst node after aggregation.
  - Aggregation = one-hot matmul: out[n,:] += sum_e Q[e,n] * (ex_e * h_src_e).
    Q (edges->nodes) and QT (nodes->edges, for broadcasting the per-dst er
    term) are graph-static one-hot fp8 tiles built on the host and streamed
    per chunk; denominators ride along as an extra tiny matmul on Q.
  - Nodes are assigned to blocks with a degree-balancing permutation so all
    blocks carry nearly equal edge counts (minimizes chunk padding).
  - Between the two GAT steps: on-device AllGather of the transposed new x
    in bf16.
"""
import os
import sys

sys.path.insert(0, "/opt/trn_rl_repo")

import numpy as np

LAST_RES = None

N = 10000
E = 320000
F = 128
H = 4
HF = H * F  # 512
NDEV = 8
NPAD = 10240
NBLK = 80
NBLK_DEV = 10
DEVN = NBLK_DEV * 128  # 1280
ROWB = 768  # bytes per table row: h fp8 @0:512, el f16 @512:520, er f16 @520:528
SENT = NPAD  # sentinel row index for pad edges
C_SHIFT = 4.0  # constant softmax shift (replaces segment_max)
NEG_SLOPE = 0.2
ONE_F8E4 = 0x38  # 1.0 in fp8 e4m3

_CACHE = {}


# ---------------------------------------------------------------- host prep
def _prep_graph(src, dst):
    # degree-balanced node -> slot permutation: assign nodes to the 80 blocks
    # so per-block edge counts are nearly equal (minimizes chunk padding).
    deg = np.bincount(dst, minlength=N)
    order_n = np.argsort(-deg, kind="stable")
    blk_load = np.zeros(NBLK, np.int64)
    blk_fill = np.zeros(NBLK, np.int64)
    slot = np.empty(NPAD, np.int64)
    node_of_slot = np.full(NPAD, -1, np.int64)
    import heapq
    heap = [(0, 0, b) for b in range(NBLK)]
    heapq.heapify(heap)
    for n in order_n:
        while True:
            load, fill, b = heapq.heappop(heap)
            if fill < 128 and fill == blk_fill[b] and load == blk_load[b]:
                break
        s = 128 * b + fill
        slot[n] = s
        node_of_slot[s] = n
        blk_load[b] += deg[n]
        blk_fill[b] += 1
        if blk_fill[b] < 128:
            heapq.heappush(heap, (int(blk_load[b]), int(blk_fill[b]), b))
    # pad nodes (no edges) fill remaining slots
    free_slots = np.where(node_of_slot < 0)[0]
    for s, vn in zip(free_slots, range(N, N + len(free_slots))):
        node_of_slot[s] = vn
    sdst = slot[dst]
    ssrc = slot[src]
    order = np.argsort(sdst, kind="stable")
    s_src = ssrc[order]
    s_dst = sdst[order]
    blk = s_dst // 128
    counts = np.bincount(blk, minlength=NBLK)
    maxcnt = int(counts.max())
    nch = max(2, 2 * ((maxcnt + 255) // 256))  # even chunk count per block
    ebpad = nch * 128
    nhalf = nch // 2
    nipc = nhalf * 128  # idxs per gather call

    starts = np.zeros(NBLK + 1, np.int64)
    np.cumsum(counts, out=starts[1:])

    # per-block padded arrays
    gidx = np.full((NBLK, ebpad), SENT, np.int64)
    dloc = np.zeros((NBLK, ebpad), np.int64)
    for b in range(NBLK):
        lo, hi = starts[b], starts[b + 1]
        cnt = hi - lo
        gidx[b, :cnt] = s_src[lo:hi]
        dloc[b, :cnt] = s_dst[lo:hi] - 128 * b

    per_core = []
    for d in range(NDEV):
        bsl = slice(NBLK_DEV * d, NBLK_DEV * (d + 1))
        g = gidx[bsl]  # [10, ebpad]
        dl = dloc[bsl]

        # big gather idx tiles: [10*2 calls, 128, nipc//16] int16
        bigidx = np.zeros((NBLK_DEV * 2, 128, nipc // 16), np.int16)
        gi = g.reshape(NBLK_DEV, 2, nipc)
        for b in range(NBLK_DEV):
            for hf_ in range(2):
                v = gi[b, hf_]  # [nipc]
                t = v.reshape(nipc // 16, 16).T.astype(np.int16)  # [16, cols]
                bigidx[2 * b + hf_] = np.tile(t, (8, 1))

        # er gather idxs: all own rows d*1280 .. d*1280+1280 -> [128, 80]
        v = (DEVN * d + np.arange(DEVN)).astype(np.int16)
        eridx = np.tile(v.reshape(DEVN // 16, 16).T, (8, 1))

        # packed one-hot tiles [10*nch, 128, 256] fp8e4m3 (as uint8 bytes):
        #   cols 0:128  = QT:  qtq[k, n, e]    = dloc(k, e) == n
        #   cols 128:256 = Q:  qtq[k, e, 128+n] = dloc(k, e) == n
        dl3 = dl.reshape(NBLK_DEV, nch, 128)  # [b, c, p]
        nk = NBLK_DEV * nch
        qtq = np.zeros((nk, 128, 256), np.uint8)
        ch_idx = np.repeat(np.arange(nk), 128)
        p_idx = np.tile(np.arange(128), nk)
        n_idx = dl3.reshape(-1)
        qtq[ch_idx, n_idx, p_idx] = ONE_F8E4
        qtq[ch_idx, p_idx, 128 + n_idx] = ONE_F8E4

        per_core.append(dict(bigidx=bigidx, eridx=eridx, qtq=qtq))
    return per_core, nch, slot, node_of_slot


def _build(nch, alpha):
    import concourse.bass as bass
    import concourse.tile as tile
    from concourse import bacc, mybir

    f32 = mybir.dt.float32
    f32r = mybir.dt.float32r
    f16 = mybir.dt.float16
    bf16 = mybir.dt.bfloat16
    f8 = mybir.dt.float8e4
    i16 = mybir.dt.int16
    u8 = mybir.dt.uint8
    nhalf = nch // 2
    nipc = nhalf * 128
    icols = nipc // 16
    CA = float((1.0 - alpha) / H)

    nc = bacc.Bacc("TRN2", target_bir_lowering=False, debug=False,
                   num_devices=NDEV, num_swdge_queues=4)

    # ---- params (shared across cores unless noted)
    xT0_p = nc.declare_dram_parameter("xT0", [128, NPAD], bf16, isOutput=False)
    W_p = nc.declare_dram_parameter("Wm", [128, HF], f32, isOutput=False)
    ALR_p = nc.declare_dram_parameter("ALR", [128, 2 * H], f32, isOutput=False)
    x0b_p = nc.declare_dram_parameter("x0b", [DEVN, F], f32, isOutput=False)  # per-core
    ident_p = nc.declare_dram_parameter("ident32", [128, 128], f32, isOutput=False)
    bigidx_p = nc.declare_dram_parameter("bigidx", [NBLK_DEV * 2, 128, icols], i16, isOutput=False)  # per-core
    eridx_p = nc.declare_dram_parameter("eridx", [128, DEVN // 16], i16, isOutput=False)  # per-core
    qtq_p = nc.declare_dram_parameter("qtq", [NBLK_DEV * nch, 128, 256], u8, isOutput=False)  # per-core
    sent_p = nc.declare_dram_parameter("sentrow", [1, ROWB], u8, isOutput=False)
    out_p = nc.declare_dram_parameter("outx", [DEVN, F], f32, isOutput=True)  # per-core

    # ---- internal DRAM
    h_table = nc.dram_tensor("h_table", [NPAD + 16, ROWB], u8)
    xt_own0 = nc.dram_tensor("xt_own0", [128, 6 * 128], bf16)
    xt_own1 = nc.dram_tensor("xt_own1", [128, 4 * 128], bf16)
    ag0 = nc.dram_tensor("ag0", [NDEV, 128, 6 * 128], bf16, addr_space="Shared")
    ag1 = nc.dram_tensor("ag1", [NDEV, 128, 4 * 128], bf16, addr_space="Shared")

    # queue 0's Q7 pair appears to host the Pool sequencer: gathers on it hold
    # the engine for their full descriptor-generation time, while queues 1-3
    # retire fast and generate asynchronously. Rotate over 1..3 only.
    gq = [0]

    def next_q():
        q = gq[0]
        gq[0] = (gq[0] + 1) % 3
        return 1 + q

    from contextlib import ExitStack
    with tile.TileContext(nc) as tc, ExitStack() as ctx:
        cpool = ctx.enter_context(tc.tile_pool(name="consts", bufs=1))
        gpool = ctx.enter_context(tc.tile_pool(name="gather", bufs=8))
        stpool = ctx.enter_context(tc.tile_pool(name="stage", bufs=4))
        xtpool = ctx.enter_context(tc.tile_pool(name="xt", bufs=4))
        qtpool = ctx.enter_context(tc.tile_pool(name="qt", bufs=6))
        mpool = ctx.enter_context(tc.tile_pool(name="msg", bufs=6))
        apool = ctx.enter_context(tc.tile_pool(name="attn", bufs=6))
        epool = ctx.enter_context(tc.tile_pool(name="epi", bufs=2))
        pbig = ctx.enter_context(tc.tile_pool(name="pbig", bufs=3, space="PSUM"))
        psm = ctx.enter_context(tc.tile_pool(name="psm", bufs=3, space="PSUM"))
        per = ctx.enter_context(tc.tile_pool(name="per", bufs=2, space="PSUM"))

        # ---- load constants
        W_sb = cpool.tile([128, HF], f32, tag="W")
        nc.sync.dma_start(out=W_sb[:], in_=W_p[:])
        W_r = cpool.tile([128, HF], bf16, tag="Wr")
        nc.vector.tensor_copy(out=W_r[:], in_=W_sb[:])
        ALR_sb = cpool.tile([128, 2 * H], f32, tag="ALR")
        nc.sync.dma_start(out=ALR_sb[:], in_=ALR_p[:])
        ALR_r = cpool.tile([128, 2 * H], bf16, tag="ALRr")
        nc.vector.tensor_copy(out=ALR_r[:], in_=ALR_sb[:])
        ident_sb = cpool.tile([128, 128], f32, tag="ident")
        nc.sync.dma_start(out=ident_sb[:], in_=ident_p[:])
        identB = cpool.tile([128, 128], f16, tag="identB")
        nc.vector.tensor_copy(out=identB[:], in_=ident_sb[:])
        bigidx_sb = cpool.tile([128, NBLK_DEV * 2 * icols], i16, tag="bigidx")
        for k in range(NBLK_DEV * 2):
            nc.sync.dma_start(
                out=bigidx_sb[:, k * icols:(k + 1) * icols], in_=bigidx_p[k]
            )
        eridx_sb = cpool.tile([128, DEVN // 16], i16, tag="eridx")
        nc.sync.dma_start(out=eridx_sb[:], in_=eridx_p[:])
        shift_sb = cpool.tile([128, 1], f32, tag="shift")
        nc.vector.memset(shift_sb[:], -C_SHIFT)
        sent_sb = cpool.tile([1, ROWB], u8, tag="sent")
        nc.sync.dma_start(out=sent_sb[:], in_=sent_p[:])
        nc.sync.dma_start(out=h_table[SENT:SENT + 1, :], in_=sent_sb[:])

        for step in range(2):
            # ================= H phase: build h_ext table for all nodes
            for c4 in range(NBLK // 4):
                xt4 = xtpool.tile([128, 4, 128], bf16, tag="xt")
                if step == 0:
                    nc.sync.dma_start(
                        out=xt4[:], in_=xT0_p[:, 512 * c4:512 * (c4 + 1)].rearrange(
                            "p (c q) -> p c q", c=4)
                    )
                else:
                    # 4 consecutive chunks may straddle an ag rank boundary only
                    # when NBLK_DEV % 4 != 0; NBLK_DEV=10 -> straddles every other
                    # group, so split into two 2-chunk loads (2 | 2).
                    for half2 in range(2):
                        c2 = 4 * c4 + 2 * half2
                        r2, cc2 = c2 // NBLK_DEV, c2 % NBLK_DEV
                        if cc2 < 6:
                            ag_src = ag0[r2, :, 128 * cc2:128 * (cc2 + 2)]
                        else:
                            ag_src = ag1[r2, :, 128 * (cc2 - 6):128 * (cc2 - 4)]
                        nc.sync.dma_start(
                            out=xt4[:, 2 * half2:2 * half2 + 2, :],
                            in_=ag_src.rearrange("p (c q) -> p c q", c=2),
                        )
                for j2 in range(2):
                    stage = stpool.tile([128, 2, ROWB], u8, tag="stage")
                    for jj in range(2):
                        c = 4 * c4 + 2 * j2 + jj
                        xt_sl = xt4[:, 2 * j2 + jj, :]
                        h_ps = pbig.tile([128, HF], f32, tag="big")
                        nc.tensor.matmul(out=h_ps[:], lhsT=xt_sl, rhs=W_r[:],
                                         start=True, stop=True)
                        e_ps = psm.tile([128, 128], f32, tag="sm")
                        nc.tensor.matmul(
                            out=e_ps[:, 0:2 * H], lhsT=xt_sl, rhs=ALR_r[:],
                            start=True, stop=True
                        )
                        if jj == 0:
                            nc.scalar.activation(
                                out=stage[:, jj, 0:HF].bitcast(f8), in_=h_ps[:],
                                func=mybir.ActivationFunctionType.Copy
                            )
                        else:
                            nc.vector.tensor_copy(
                                out=stage[:, jj, 0:HF].bitcast(f8), in_=h_ps[:]
                            )
                        nc.vector.tensor_copy(
                            out=stage[:, jj, HF:HF + 16].bitcast(f16), in_=e_ps[:, 0:2 * H]
                        )
                    c0 = 4 * c4 + 2 * j2
                    nc.sync.dma_start(
                        out=h_table[128 * c0:128 * (c0 + 2), :].rearrange(
                            "(j p) w -> p j w", p=128),
                        in_=stage[:],
                    )

            # ================= AGG phase: own blocks
            erg = epool.tile([128, NBLK_DEV, 256], u8, tag="erg")
            nc.gpsimd.dma_gather(
                out_ap=erg[:],
                in_ap=h_table[:, HF:HF + 256],
                idxs_ap=eridx_sb[:],
                num_idxs=DEVN,
                num_idxs_reg=DEVN,
                elem_size=256,
                elem_step=ROWB,
                single_packet=False,
                queue_num=next_q(),
            )
            def attention(b):
                halves = []
                for hf_ in range(2):
                    G = gpool.tile([128, nhalf, ROWB], u8, tag="G")
                    call = 2 * b + hf_
                    nc.gpsimd.dma_gather(
                        out_ap=G[:],
                        in_ap=h_table[0:NPAD + 16, 0:ROWB],
                        idxs_ap=bigidx_sb[:, call * icols:(call + 1) * icols],
                        num_idxs=nipc,
                        num_idxs_reg=nipc,
                        elem_size=ROWB,
                        elem_step=ROWB,
                        single_packet=False,
                        queue_num=next_q(),
                    )
                    k0 = b * nch + hf_ * nhalf
                    qtq_t = qtpool.tile([128, nhalf, 256], u8, tag="qt")
                    nc.sync.dma_start(
                        out=qtq_t[:],
                        in_=qtq_p[k0:k0 + nhalf].rearrange("c p w -> p c w"),
                    )
                    er_ps = per.tile([128, 4 * nhalf], f32, tag="er")
                    for cc in range(nhalf):
                        nc.tensor.matmul(
                            out=er_ps[:, 4 * cc:4 * cc + 4],
                            lhsT=qtq_t[:, cc, 0:128].bitcast(f8),
                            rhs=erg[:, b, 8:16].bitcast(f16),
                            start=True, stop=True,
                        )
                    z = apool.tile([128, 4 * nhalf], f32, tag="z")
                    nc.vector.tensor_tensor(
                        out=z[:], in0=G[:, :, HF:HF + 8].bitcast(f16), in1=er_ps[:],
                        op=mybir.AluOpType.add
                    )
                    v = apool.tile([128, 4 * nhalf], f32, tag="v")
                    nc.vector.tensor_scalar(
                        out=v[:], in0=z[:], scalar1=NEG_SLOPE, scalar2=None,
                        op0=mybir.AluOpType.mult,
                    )
                    w = apool.tile([128, 4 * nhalf], f32, tag="w")
                    nc.vector.tensor_tensor(
                        out=w[:], in0=z[:], in1=v[:], op=mybir.AluOpType.max
                    )
                    ex32 = apool.tile([128, 4 * nhalf], f32, tag="ex32")
                    nc.scalar.activation(
                        out=ex32[:], in_=w[:], func=mybir.ActivationFunctionType.Exp,
                        bias=shift_sb[:, 0:1],
                    )
                    ex16 = apool.tile([128, 4 * nhalf], f16, tag="ex")
                    nc.scalar.activation(
                        out=ex16[:], in_=ex32[:],
                        func=mybir.ActivationFunctionType.Copy,
                    )
                    halves.append((G, qtq_t, ex16, ex32))
                return halves

            def aggregation(b, halves, step):
                out_ps = pbig.tile([128, HF], f32, tag="big")
                den_ps = psm.tile([128, 128], f32, tag="sm")
                for hf_, (G, qtq_t, ex16, ex32) in enumerate(halves):
                    # denominator matmuls first: their rhs (ex16) is ready early
                    for cc in range(nhalf):
                        cg = hf_ * nhalf + cc
                        nc.tensor.matmul(
                            out=den_ps[:, 0:H], lhsT=qtq_t[:, cc, 128:256].bitcast(f8),
                            rhs=ex16[:, 4 * cc:4 * cc + 4],
                            start=(cg == 0), stop=(cg == nch - 1), skip_group_check=True,
                        )
                    for cc in range(nhalf):
                        cg = hf_ * nhalf + cc
                        msg = mpool.tile([128, H, F], f16, tag="msg")
                        if cg % 3 != 2:
                            # DVE path: broadcast multiply (~600 ns)
                            nc.vector.tensor_tensor(
                                out=msg[:],
                                in0=G[:, cc, 0:HF].bitcast(f8).rearrange(
                                    "p (h f) -> p h f", h=H),
                                in1=ex16[:, 4 * cc:4 * cc + 4, None].to_broadcast([128, H, F]),
                                op=mybir.AluOpType.mult,
                            )
                        else:
                            # Act path: per-head scaled copies; for a fixed
                            # head the ex factor is per-partition (per-edge),
                            # which is Act's native scale operand
                            for hd in range(H):
                                nc.scalar.activation(
                                    out=msg[:, hd],
                                    in_=G[:, cc, F * hd:F * (hd + 1)].bitcast(f8),
                                    func=mybir.ActivationFunctionType.Copy,
                                    scale=ex32[:, 4 * cc + hd:4 * cc + hd + 1],
                                )
                        nc.tensor.matmul(
                            out=out_ps[:], lhsT=qtq_t[:, cc, 128:256].bitcast(f8),
                            rhs=msg[:].rearrange("p h f -> p (h f)"),
                            start=(cg == 0), stop=(cg == nch - 1), skip_group_check=True,
                        )

                # ---- epilogue for block b
                den_sb = epool.tile([128, H], f32, tag="den")
                nc.vector.tensor_scalar(
                    out=den_sb[:], in0=den_ps[:, 0:H], scalar1=1e-30, scalar2=None,
                    op0=mybir.AluOpType.add,
                )
                rden = epool.tile([128, H], f32, tag="rden")
                nc.vector.reciprocal(out=rden[:], in_=den_sb[:])
                ms = []
                for hd in range(H):
                    m = epool.tile([128, F], f32, tag=f"m{hd}")
                    nc.scalar.activation(
                        out=m[:], in_=out_ps[:, F * hd:F * (hd + 1)],
                        func=mybir.ActivationFunctionType.Copy,
                        scale=rden[:, hd:hd + 1],
                    )
                    ms.append(m)
                a01 = epool.tile([128, F], f32, tag="a01")
                nc.vector.tensor_tensor(out=a01[:], in0=ms[0][:], in1=ms[1][:], op=mybir.AluOpType.add)
                a23 = epool.tile([128, F], f32, tag="a23")
                nc.vector.tensor_tensor(out=a23[:], in0=ms[2][:], in1=ms[3][:], op=mybir.AluOpType.add)
                macc = epool.tile([128, F], f32, tag="macc")
                nc.vector.tensor_tensor(out=macc[:], in0=a01[:], in1=a23[:], op=mybir.AluOpType.add)
                x0b_t = epool.tile([128, F], f32, tag="x0b")
                nc.sync.dma_start(out=x0b_t[:], in_=x0b_p[128 * b:128 * (b + 1), :])
                sc = epool.tile([128, F], f32, tag="sc")
                nc.vector.tensor_scalar(
                    out=sc[:], in0=macc[:], scalar1=CA, scalar2=None, op0=mybir.AluOpType.mult
                )
                outf = epool.tile([128, F], f32, tag="outf")
                nc.vector.tensor_tensor(out=outf[:], in0=sc[:], in1=x0b_t[:], op=mybir.AluOpType.add)
                if step == 0:
                    tp_ps = psm.tile([128, 128], f32, tag="sm")
                    nc.tensor.transpose(out=tp_ps[:], in_=outf[:], identity=ident_sb[:])
                    xtb = epool.tile([128, 128], bf16, tag="xtb")
                    nc.vector.tensor_copy(out=xtb[:], in_=tp_ps[:])
                    if b < 6:
                        nc.sync.dma_start(out=xt_own0[:, 128 * b:128 * (b + 1)], in_=xtb[:])
                    else:
                        nc.sync.dma_start(out=xt_own1[:, 128 * (b - 6):128 * (b - 5)], in_=xtb[:])
                    if b == 5:
                        # first 6 blocks done on every rank: gather them while
                        # the last 4 blocks are still aggregating
                        nc.gpsimd.collective_compute(
                            "AllGather",
                            bass.mybir.AluOpType.bypass,
                            replica_groups=[list(range(NDEV))],
                            ins=[xt_own0[:]],
                            outs=[ag0[:]],
                        )
                else:
                    nc.sync.dma_start(out=out_p[128 * b:128 * (b + 1), :], in_=outf[:])

            # software pipeline: attention for block b+1 is issued before the
            # heavy aggregation matmuls of block b so the tensor/DVE/Act
            # streams of consecutive blocks overlap
            pend = attention(0)
            for b in range(NBLK_DEV):
                nxt = attention(b + 1) if b + 1 < NBLK_DEV else None
                aggregation(b, pend, step)
                pend = nxt

            if step == 0:
                nc.gpsimd.collective_compute(
                    "AllGather",
                    bass.mybir.AluOpType.bypass,
                    replica_groups=[list(range(NDEV))],
                    ins=[xt_own1[:]],
                    outs=[ag1[:]],
                )

    nc.compile()
    return nc


# ---------------------------------------------------------------- entry point
def kernel(x, x0, src, dst, alpha, W, attn_l, attn_r, bias):
    x = np.asarray(x, np.float32)
    x0 = np.asarray(x0, np.float32)
    src = np.asarray(src).astype(np.int64)
    dst = np.asarray(dst).astype(np.int64)
    alpha = float(np.asarray(alpha))
    W = np.asarray(W, np.float32)
    attn_l = np.asarray(attn_l, np.float32)
    attn_r = np.asarray(attn_r, np.float32)
    bias = np.asarray(bias, np.float32)

    per_core, nch, slot, node_of_slot = _prep_graph(src, dst)

    key = (nch, round(alpha, 9))
    if key not in _CACHE:
        _CACHE[key] = _build(nch, alpha)
    nc = _CACHE[key]

    # shared host inputs
    xpad = np.zeros((NPAD, F), np.float32)
    real = node_of_slot < N
    xpad[real] = x[node_of_slot[real]]
    import ml_dtypes
    xT0 = np.ascontiguousarray(xpad.T).astype(ml_dtypes.bfloat16)  # [128, NPAD]
    ALR = np.zeros((128, 2 * H), np.float32)
    Wr = W.reshape(F, H, F)
    ALR[:, 0:H] = np.einsum("fhg,hg->fh", Wr, attn_l)
    ALR[:, H:2 * H] = np.einsum("fhg,hg->fh", Wr, attn_r)
    ident32 = np.eye(128, dtype=np.float32)
    bias_mean = bias.mean(axis=0)  # [F]
    x0b_full = np.zeros((NPAD, F), np.float32)
    x0b_full[real] = alpha * x0[node_of_slot[real]] + (1.0 - alpha) * bias_mean[None, :]
    sentrow = np.zeros((1, ROWB), np.uint8)
    sentrow[0, HF:HF + 8] = np.full(4, -60000.0, np.float16).view(np.uint8)

    from concourse.bass_utils import run_bass_kernel_spmd

    in_maps = []
    for d in range(NDEV):
        pc = per_core[d]
        in_maps.append({
            "xT0": xT0, "Wm": W, "ALR": ALR,
            "x0b": x0b_full[DEVN * d:DEVN * (d + 1)],
            "ident32": ident32, "bigidx": pc["bigidx"],
            "eridx": pc["eridx"], "qtq": pc["qtq"], "sentrow": sentrow,
        })
    global LAST_RES
    res = None
    for attempt in range(3):
        try:
            res = run_bass_kernel_spmd(
                nc, in_maps, list(range(NDEV)),
                trace=bool(os.environ.get("GAT_TRACE")),
            )
            break
        except Exception:
            if attempt == 2:
                raise
            import time as _time
            _time.sleep(2.0)
    LAST_RES = res
    out_slots = np.concatenate([r["outx"] for r in res.results], axis=0)
    return out_slots[slot[np.arange(N)]].astype(np.float32)


if __name__ == "__main__":
    rng = np.random.default_rng(0)
    x = rng.standard_normal((N, F), dtype=np.float32)
    x0 = rng.standard_normal((N, F), dtype=np.float32)
    src = rng.integers(0, N, E).astype(np.int32)
    dst = rng.integers(0, N, E).astype(np.int32)
    W = (rng.standard_normal((F, H * F)).astype(np.float32) / np.sqrt(F))
    al = (rng.standard_normal((H, F)).astype(np.float32) / np.sqrt(F))
    ar = (rng.standard_normal((H, F)).astype(np.float32) / np.sqrt(F))
    bias = np.zeros((H, F), np.float32)
    out = kernel(x=x, x0=x0, src=src, dst=dst, alpha=np.float32(0.1),
                 W=W, attn_l=al, attn_r=ar, bias=bias)
    print("out", out.shape, out.dtype, float(np.abs(out).max()))
